# revision 21
# baseline (speedup 1.0000x reference)
"""DepLabeledGCN Trainium2 kernel — data-parallel, fp8e3 layer-1 weights.

Each core processes ITS OWN batch with ALL 48 label matrices (no
collectives; B=8 == 8 cores).

Key structure (per layer):
    s-phase:  sT[l,kc] chunks = per-label masked-adjacency matmuls,
              label pairs fused into N=256 matmuls; PSUM->SBUF copies
              split across Vector(l2=0)/Scalar(l2=1) to halve latency
    msum:     msg = sum_{l,kc} sT[l,kc] @ W_l^T[kc], 192 accumulating
              matmuls into one PSUM bank
    relu(msg * 1/denom) -> next h, emitted in 4 kc-chunks (V/ACT)

Precision/bandwidth plan: layer-1 weights are stored in HBM as
fp8e3 (e3m4) scaled by 32 (12.6 MB, fully SBUF-resident, used directly
as the MOVING matmul operand against fp16 sT — mixed-dtype matmul).
The 1/32 dequant scale folds into layer-1's 1/denom. Layer-2 weights
stream as true fp16 (25.2 MB) through an 8-deep rotating pool while
layers 1+2 compute; measured end-to-end rel err ≈ 1.2e-2 on the
deterministic setup_inputs() (gate 2e-2). A short dummy-matmul burst
warms the PE HAM clock gate before the first real matmuls.
"""

import sys

if '/opt/trn_rl_repo' not in sys.path:
    sys.path.insert(0, '/opt/trn_rl_repo')

import numpy as np
import ml_dtypes

B, N, D, L = 8, 128, 512, 48
NCORES = 8
KC = D // 128
NUM_LAYERS = 2
NP = L // 2             # label pairs per layer
W8_SCALE = 32.0         # fp8e3 weight pre-scale (power of two)
WST_BUFS = 10           # layer-2 fp16 weight rotating pool depth
N_FP8_L2 = 3            # trailing layer-2 pairs that reuse the fp8 weights
N_WARM = 30             # dummy warm-up matmuls for the HAM clock gate
                        # (needs ~3.4us of sustained PE activity to unthrottle)

_CACHE = {}


def _build_nc():
    import concourse.bass as bass
    import concourse.mybir as mybir
    import concourse.tile as tile
    from concourse import bacc
    from concourse.masks import make_identity

    dt = mybir.dt
    f32 = dt.float32
    f16 = dt.float16
    f8 = dt.float8e3
    Alu = mybir.AluOpType

    nc = bacc.Bacc("TRN2", target_bir_lowering=False, debug=False,
                   num_devices=NCORES)

    gcn_e = nc.dram_tensor("gcn16", [128, D], f16, kind="ExternalInput").ap()
    adjT_e = nc.dram_tensor("adjT", [N, N], f32, kind="ExternalInput").ap()
    labT_e = nc.dram_tensor("labT", [N, N], f32, kind="ExternalInput").ap()
    adjR_e = nc.dram_tensor("adjR", [N, N], f32, kind="ExternalInput").ap()
    w8_e = nc.dram_tensor("w8", [128, L, KC, D], f8, kind="ExternalInput").ap()
    wT_e = nc.dram_tensor("wT", [128, L, KC, D], f16, kind="ExternalInput").ap()
    w0T_e = nc.dram_tensor("w0T", [128, KC, D], f16, kind="ExternalInput").ap()
    w1T_e = nc.dram_tensor("w1T", [128, KC, D], f16, kind="ExternalInput").ap()
    b0_e = nc.dram_tensor("b0", [128, KC], f32, kind="ExternalInput").ap()
    b1_e = nc.dram_tensor("b1", [128, KC], f32, kind="ExternalInput").ap()
    out_e = nc.dram_tensor("out", [128, KC, N], f32, kind="ExternalOutput").ap()

    with tile.TileContext(nc) as tc:
        with (
            tc.tile_pool(name="const", bufs=1) as cpool,
            tc.tile_pool(name="sTv", bufs=3) as sTv_pool,
            tc.tile_pool(name="sTa", bufs=3) as sTa_pool,
            tc.tile_pool(name="wst", bufs=WST_BUFS) as wst_pool,
            tc.tile_pool(name="spsum", bufs=3, space="PSUM") as spsum,
            tc.tile_pool(name="mpsum", bufs=2, space="PSUM") as mpsum,
        ):
            # -------- PE warm-up: dummy matmuls on the identity ------------
            identity = cpool.tile([128, 128], f16, tag="ident")
            make_identity(nc, identity[:])
            warm_ps = spsum.tile([128, KC, 2, 128], f32, tag="spsum",
                                 name="warm")
            for i in range(N_WARM):
                nc.tensor.matmul(warm_ps[:, 0, 0, :], lhsT=identity[:],
                                 rhs=identity[:], start=True, stop=True)

            # -------- critical-path input loads ----------------------------
            adjT_sb = cpool.tile([128, N], f32, tag="adjT")
            nc.sync.dma_start(adjT_sb[:], adjT_e)
            labT_sb = cpool.tile([128, N], f32, tag="labT")
            nc.sync.dma_start(labT_sb[:], labT_e)

            # h[0] is one tile (DMA-written); later layers are 4 per-kc
            # tiles so Vector and Scalar can write them concurrently
            # (Tile serializes multi-engine writers of a single tile).
            h0 = cpool.tile([128, D], f16, tag="h0", name="h0")
            nc.sync.dma_start(h0[:], gcn_e)
            hc = [None] + [
                [cpool.tile([128, 128], f16, tag=f"h{ly}k{kc}",
                            name=f"h{ly}k{kc}")
                 for kc in range(KC)]
                for ly in range(1, NUM_LAYERS + 1)]

            def h_chunk(ly, kc):
                if ly == 0:
                    return h0[:, kc * 128:(kc + 1) * 128]
                return hc[ly][kc][:]

            # layer-1 fp8 weights: fully resident, per-pair granularity.
            # adjR (needed only for the end-of-layer denominator) is issued
            # after the first weight pairs so it doesn't delay msum pair 0.
            w8 = cpool.tile([128, L, KC, D], f8, tag="w8")
            adjR_sb = cpool.tile([128, N], f32, tag="adjR")
            for p in range(NP):
                nc.sync.dma_start(w8[:, 2 * p:2 * p + 2], w8_e[:, 2 * p:2 * p + 2])
                if p == 1:
                    nc.sync.dma_start(adjR_sb[:], adjR_e)

            # -------- masks: maskT[j, l, i] = (labT == l) * adjT -----------
            # fp8e3 represents 0/1 exactly; mixed fp16xfp8 matmul is native
            maskT = cpool.tile([128, L, N], f8, tag="maskT")

            def emit_mask(l):
                nc.vector.scalar_tensor_tensor(
                    out=maskT[:, l, :],
                    in0=labT_sb[:],
                    scalar=float(l),
                    in1=adjT_sb[:],
                    op0=Alu.is_equal,
                    op1=Alu.mult,
                )

            # only the first two pairs upfront: anything more delays the
            # first sT copy behind them in the Vector FIFO
            for l in range(4):
                emit_mask(l)

            # -------- GCN layers -------------------------------------------
            def emit_s(ly, p):
                """s-phase for label pair p: one N=256 matmul per kc."""
                ps = spsum.tile([128, KC, 2, 128], f32, tag="spsum",
                                name="spsum")
                for kc in range(KC):
                    nc.tensor.matmul(
                        ps[:, kc, :, :],
                        lhsT=h_chunk(ly, kc),
                        rhs=maskT[:, 2 * p:2 * p + 2, :],
                        start=True, stop=True,
                    )
                # copies on separate destination tiles run truly parallel
                # on Vector/Scalar; pairs served from the fp8 weights fold
                # in the 1/32 dequant scale here
                sTv = sTv_pool.tile([128, KC, 128], f16, tag="sTv", name="sTv")
                sTa = sTa_pool.tile([128, KC, 128], f16, tag="sTa", name="sTa")
                if ly == 1 and p >= NP - N_FP8_L2:
                    nc.vector.tensor_scalar_mul(sTv[:], ps[:, :, 0, :],
                                                1.0 / W8_SCALE)
                    nc.scalar.activation(sTa[:], ps[:, :, 1, :],
                                         mybir.ActivationFunctionType.Copy,
                                         scale=1.0 / W8_SCALE)
                else:
                    nc.vector.tensor_copy(sTv[:], ps[:, :, 0, :])
                    nc.scalar.copy(sTa[:], ps[:, :, 1, :])
                return (sTv, sTa)

            def get_w2(p):
                """Layer-2 fp16 weight pair p from the rotating stream."""
                w = wst_pool.tile([128, 2, KC, D], f16, tag="wst", name="wst")
                nc.sync.dma_start(w[:], wT_e[:, 2 * p:2 * p + 2])
                return w

            first = True
            for ly in range(NUM_LAYERS):
                pm = mpsum.tile([128, D], f32, tag="mm", name="mm")
                sT_q = [emit_s(ly, 0), emit_s(ly, 1)]
                if first:
                    # denominators: emitted after the first sT copies so the
                    # Vector FIFO reaches those copies first
                    den = cpool.tile([128, 1], f32, tag="den")
                    nc.vector.tensor_reduce(den[:], adjR_sb[:],
                                            mybir.AxisListType.X, Alu.add)
                    nc.vector.tensor_scalar_add(den[:], den[:], 1.0)
                    recip = cpool.tile([128, 1], f32, tag="recip")
                    nc.vector.reciprocal(recip[:], den[:])
                    recipS = cpool.tile([128, 1], f32, tag="recipS")
                    nc.vector.tensor_scalar_mul(recipS[:], recip[:],
                                                1.0 / W8_SCALE)
                    first = False
                for p in range(NP):
                    if ly == 0 and p + 2 < NP:
                        emit_mask(2 * (p + 2))
                        emit_mask(2 * (p + 2) + 1)
                    if p + 2 < NP:
                        sT_q.append(emit_s(ly, p + 2))
                    if ly == 0:
                        w = w8[:, 2 * p:2 * p + 2]
                    elif p >= NP - N_FP8_L2:
                        w = w8[:, 2 * p:2 * p + 2]
                    else:
                        w = get_w2(p)
                    sTv, sTa = sT_q[p]
                    for l2 in range(2):
                        sT = sTv if l2 == 0 else sTa
                        for kc in range(KC):
                            i = (p * 2 + l2) * KC + kc
                            nc.tensor.matmul(
                                pm[:],
                                lhsT=sT[:, kc, :],
                                rhs=w[:, l2, kc, :],
                                start=(i == 0), stop=(i == L * KC - 1),
                            )
                # relu(msg * recip) -> next h (fp16), per-kc tiles (V/ACT)
                scl = recipS if ly == 0 else recip
                for kc in range(KC):
                    dst = hc[ly + 1][kc][:]
                    src = pm[:, kc * 128:(kc + 1) * 128]
                    if kc % 2 == 0:
                        nc.vector.tensor_scalar(dst, src, scl[:], 0.0,
                                                Alu.mult, Alu.max)
                    else:
                        nc.scalar.activation(dst, src,
                                             mybir.ActivationFunctionType.Relu,
                                             scale=scl[:])

            # -------- MLP ---------------------------------------------------
            b0_sb = cpool.tile([128, KC], f32, tag="b0")
            nc.sync.dma_start(b0_sb[:], b0_e)
            b1_sb = cpool.tile([128, KC], f32, tag="b1")
            nc.sync.dma_start(b1_sb[:], b1_e)
            w0T_sb = cpool.tile([128, KC, D], f16, tag="w0T")
            nc.sync.dma_start(w0T_sb[:], w0T_e)
            w1T_sb = cpool.tile([128, KC, D], f16, tag="w1T")
            nc.sync.dma_start(w1T_sb[:], w1T_e)
            hTc = [cpool.tile([128, 128], f16, tag=f"hT{kc}", name=f"hT{kc}")
                   for kc in range(KC)]
            pt = mpsum.tile([128, KC, 128], f16, tag="mm", name="ptr")
            for kc in range(KC):
                nc.tensor.transpose(pt[:, kc, :], h_chunk(NUM_LAYERS, kc),
                                    identity[:])
                if kc % 2 == 0:
                    nc.vector.tensor_copy(hTc[kc][:], pt[:, kc])
                else:
                    nc.scalar.copy(hTc[kc][:], pt[:, kc])

            x1c = [cpool.tile([128, 128], f16, tag=f"x1_{blk}",
                               name=f"x1_{blk}")
                   for blk in range(KC)]
            px1 = mpsum.tile([128, KC, 128], f32, tag="mm", name="px1")
            for blk in range(KC):
                for kc in range(KC):
                    nc.tensor.matmul(
                        px1[:, blk, :],
                        lhsT=w0T_sb[:, kc, blk * 128:(blk + 1) * 128],
                        rhs=hTc[kc][:],
                        start=(kc == 0), stop=(kc == KC - 1),
                    )
            # bias+relu after the full matmul stream, split V/ACT
            for blk in range(KC):
                if blk % 2 == 0:
                    nc.vector.tensor_scalar(x1c[blk][:], px1[:, blk, :],
                                            b0_sb[:, blk:blk + 1], 0.0,
                                            Alu.add, Alu.max)
                else:
                    nc.scalar.activation(x1c[blk][:], px1[:, blk, :],
                                         mybir.ActivationFunctionType.Relu,
                                         bias=b0_sb[:, blk:blk + 1])

            x2v = cpool.tile([128, 2, 128], f32, tag="x2v")
            x2a = cpool.tile([128, 2, 128], f32, tag="x2a")
            px2 = mpsum.tile([128, KC, 128], f32, tag="mm", name="px2")
            for blk in range(KC):
                for kc in range(KC):
                    nc.tensor.matmul(
                        px2[:, blk, :],
                        lhsT=w1T_sb[:, kc, blk * 128:(blk + 1) * 128],
                        rhs=x1c[kc][:],
                        start=(kc == 0), stop=(kc == KC - 1),
                    )
            for blk in range(KC):
                if blk < 2:
                    nc.vector.tensor_scalar(x2v[:, blk, :], px2[:, blk, :],
                                            b1_sb[:, blk:blk + 1], 0.0,
                                            Alu.add, Alu.max)
                else:
                    nc.scalar.activation(x2a[:, blk - 2, :], px2[:, blk, :],
                                         mybir.ActivationFunctionType.Relu,
                                         bias=b1_sb[:, blk:blk + 1])
                if blk == 1:
                    nc.sync.dma_start(out_e[:, :2], x2v[:])
            nc.sync.dma_start(out_e[:, 2:], x2a[:])

    nc.compile()
    return nc


def _get_nc():
    if "nc" not in _CACHE:
        _CACHE["nc"] = _build_nc()
    return _CACHE["nc"]


def kernel(gcn_inputs, word_seq_len, adj_matrix, dep_label_matrix,
           w_params, mlp_w0, mlp_b0, mlp_w1, mlp_b1, **_unused):
    from concourse.bass_utils import run_bass_kernel_spmd

    gcn = np.asarray(gcn_inputs, dtype=np.float32)
    adj = np.asarray(adj_matrix, dtype=np.float32)
    lab = np.asarray(dep_label_matrix)
    w = np.asarray(w_params, dtype=np.float32)
    w0 = np.asarray(mlp_w0, dtype=np.float32)
    w1 = np.asarray(mlp_w1, dtype=np.float32)
    b0 = np.asarray(mlp_b0, dtype=np.float32)
    b1 = np.asarray(mlp_b1, dtype=np.float32)

    # wT[kmod, l, kc, d] = w[l, d, kc*128+kmod]  (shared by all cores)
    wT = w.transpose(0, 2, 1).reshape(L, KC, 128, D).transpose(2, 0, 1, 3)
    wT = np.ascontiguousarray(wT)
    wT16 = wT.astype(np.float16)
    w8 = (wT * W8_SCALE).astype(ml_dtypes.float8_e3m4)
    w0T = np.ascontiguousarray(
        w0.T.reshape(KC, 128, D).transpose(1, 0, 2)).astype(np.float16)
    w1T = np.ascontiguousarray(
        w1.T.reshape(KC, 128, D).transpose(1, 0, 2)).astype(np.float16)
    b0r = np.ascontiguousarray(b0.reshape(KC, 128).T)
    b1r = np.ascontiguousarray(b1.reshape(KC, 128).T)
    labf = lab.astype(np.float32)

    in_maps = []
    for c in range(NCORES):
        in_maps.append({
            "gcn16": gcn[c].astype(np.float16),
            "adjT": np.ascontiguousarray(adj[c].T),
            "labT": np.ascontiguousarray(labf[c].T),
            "adjR": np.ascontiguousarray(adj[c]),
            "w8": w8,
            "wT": wT16,
            "w0T": w0T,
            "w1T": w1T,
            "b0": b0r,
            "b1": b1r,
        })

    nc = _get_nc()
    res = run_bass_kernel_spmd(nc, in_maps, list(range(NCORES)))

    out = np.empty((B, N, D), dtype=np.float32)
    for c in range(NCORES):
        arr = res.results[c]["out"]          # [dmod, blk, i]
        out[c] = np.transpose(arr, (2, 1, 0)).reshape(N, D)
    return out


# revision 22
# speedup vs baseline: 1.0230x; 1.0230x over previous
"""DepLabeledGCN Trainium2 kernel — data-parallel, fp8e3 layer-1 weights.

Each core processes ITS OWN batch with ALL 48 label matrices (no
collectives; B=8 == 8 cores).

Key structure (per layer):
    s-phase:  sT[l,kc] chunks = per-label masked-adjacency matmuls,
              label pairs fused into N=256 matmuls
    msum:     msg = sum_{l,kc} sT[l,kc] @ W_l^T[kc], 192 accumulating
              matmuls into one PSUM bank
    relu(msg * 1/denom) -> next h

Precision/bandwidth plan: layer-1 weights are stored in HBM as
fp8e3 (e3m4) scaled by 32 (12.6 MB, fully SBUF-resident, used directly
as the MOVING matmul operand against fp16 sT — mixed-dtype matmul).
The 1/32 dequant scale folds into layer-1's 1/denom. Layer-2 weights
stream as true fp16 through a 10-deep rotating pool while layers 1+2
compute; the last N_FP8_L2 pairs of layer 2 reuse the resident fp8
weights (1/32 folded into their sT copies) so the fp16 stream finishes
well before its consumption deadline. Measured end-to-end rel err
≈ 1.4e-2 on the deterministic setup_inputs() (gate 2e-2). A dummy
matmul burst warms the PE HAM clock gate before the first real work.
"""

import sys

if '/opt/trn_rl_repo' not in sys.path:
    sys.path.insert(0, '/opt/trn_rl_repo')

import numpy as np
import ml_dtypes

B, N, D, L = 8, 128, 512, 48
NCORES = 8
KC = D // 128
NUM_LAYERS = 2
NP = L // 2             # label pairs per layer
W8_SCALE = 32.0         # fp8e3 weight pre-scale (power of two)
WST_BUFS = 10           # layer-2 fp16 weight rotating pool depth
N_FP8_L2 = 5            # trailing layer-2 pairs that reuse the fp8 weights
N_WARM = 30             # dummy warm-up matmuls for the HAM clock gate
                        # (needs ~3.4us of sustained PE activity to unthrottle)

_CACHE = {}


def _build_nc():
    import concourse.bass as bass
    import concourse.mybir as mybir
    import concourse.tile as tile
    from concourse import bacc
    from concourse.masks import make_identity

    dt = mybir.dt
    f32 = dt.float32
    f16 = dt.float16
    f8 = dt.float8e3
    Alu = mybir.AluOpType
    Act = mybir.ActivationFunctionType

    nc = bacc.Bacc("TRN2", target_bir_lowering=False, debug=False,
                   num_devices=NCORES)

    gcn_e = nc.dram_tensor("gcn16", [128, D], f16, kind="ExternalInput").ap()
    adjT_e = nc.dram_tensor("adjT", [N, N], f32, kind="ExternalInput").ap()
    labT_e = nc.dram_tensor("labT", [N, N], f32, kind="ExternalInput").ap()
    adjR_e = nc.dram_tensor("adjR", [N, N], f32, kind="ExternalInput").ap()
    w8_e = nc.dram_tensor("w8", [128, L, KC, D], f8, kind="ExternalInput").ap()
    wT_e = nc.dram_tensor("wT", [128, L, KC, D], f16, kind="ExternalInput").ap()
    w0T_e = nc.dram_tensor("w0T", [128, KC, D], f16, kind="ExternalInput").ap()
    w1T_e = nc.dram_tensor("w1T", [128, KC, D], f16, kind="ExternalInput").ap()
    b0_e = nc.dram_tensor("b0", [128, KC], f32, kind="ExternalInput").ap()
    b1_e = nc.dram_tensor("b1", [128, KC], f32, kind="ExternalInput").ap()
    out_e = nc.dram_tensor("out", [128, KC, N], f32, kind="ExternalOutput").ap()

    with tile.TileContext(nc) as tc:
        with (
            tc.tile_pool(name="const", bufs=1) as cpool,
            tc.tile_pool(name="sT", bufs=3) as sT_pool,
            tc.tile_pool(name="wst", bufs=WST_BUFS) as wst_pool,
            tc.tile_pool(name="spsum", bufs=3, space="PSUM") as spsum,
            tc.tile_pool(name="mpsum", bufs=2, space="PSUM") as mpsum,
        ):
            # -------- PE warm-up: dummy matmuls on the identity ------------
            identity = cpool.tile([128, 128], f16, tag="ident")
            make_identity(nc, identity[:])
            warm_ps = spsum.tile([128, KC, 2, 128], f32, tag="spsum",
                                 name="warm")
            for i in range(N_WARM):
                nc.tensor.matmul(warm_ps[:, 0, 0, :], lhsT=identity[:],
                                 rhs=identity[:], start=True, stop=True)

            # -------- critical-path input loads ----------------------------
            adjT_sb = cpool.tile([128, N], f32, tag="adjT")
            nc.sync.dma_start(adjT_sb[:], adjT_e)
            labT_sb = cpool.tile([128, N], f32, tag="labT")
            nc.sync.dma_start(labT_sb[:], labT_e)

            h = [cpool.tile([128, D], f16, tag=f"h{ly}", name=f"h{ly}")
                 for ly in range(NUM_LAYERS + 1)]
            nc.sync.dma_start(h[0][:], gcn_e)

            # layer-1 fp8 weights: fully resident, per-pair granularity.
            # adjR (needed only for the end-of-layer denominator) is issued
            # after the first weight pairs so it doesn't delay msum pair 0.
            w8 = cpool.tile([128, L, KC, D], f8, tag="w8")
            adjR_sb = cpool.tile([128, N], f32, tag="adjR")
            for p in range(NP):
                nc.sync.dma_start(w8[:, 2 * p:2 * p + 2],
                                  w8_e[:, 2 * p:2 * p + 2])
                if p == 1:
                    nc.sync.dma_start(adjR_sb[:], adjR_e)

            # -------- masks: maskT[j, l, i] = (labT == l) * adjT -----------
            # fp8e3 represents 0/1 exactly; mixed fp16xfp8 matmul is native
            maskT = cpool.tile([128, L, N], f8, tag="maskT")

            def emit_mask(l):
                nc.vector.scalar_tensor_tensor(
                    out=maskT[:, l, :],
                    in0=labT_sb[:],
                    scalar=float(l),
                    in1=adjT_sb[:],
                    op0=Alu.is_equal,
                    op1=Alu.mult,
                )

            # only the first two pairs upfront: anything more delays the
            # first sT copy behind them in the Vector FIFO
            for l in range(4):
                emit_mask(l)

            # -------- GCN layers -------------------------------------------
            def emit_s(ly, p):
                """s-phase for label pair p: one N=256 matmul per kc."""
                ps = spsum.tile([128, KC, 2, 128], f32, tag="spsum",
                                name="spsum")
                for kc in range(KC):
                    nc.tensor.matmul(
                        ps[:, kc, :, :],
                        lhsT=h[ly][:, kc * 128:(kc + 1) * 128],
                        rhs=maskT[:, 2 * p:2 * p + 2, :],
                        start=True, stop=True,
                    )
                sT = sT_pool.tile([128, 2, KC, 128], f16, tag="sT", name="sT")
                srcp = ps.rearrange("q kc l i -> q l kc i")
                # pairs served from the fp8 weights fold in the 1/32 dequant
                if ly == 1 and p >= NP - N_FP8_L2:
                    nc.vector.tensor_scalar_mul(sT[:, 0], srcp[:, 0],
                                                1.0 / W8_SCALE)
                    nc.scalar.activation(sT[:, 1], srcp[:, 1], Act.Copy,
                                         scale=1.0 / W8_SCALE)
                else:
                    nc.vector.tensor_copy(sT[:, 0], srcp[:, 0])
                    nc.scalar.copy(sT[:, 1], srcp[:, 1])
                return sT

            def get_w2(p):
                """Layer-2 fp16 weight pair p from the rotating stream."""
                w = wst_pool.tile([128, 2, KC, D], f16, tag="wst", name="wst")
                nc.sync.dma_start(w[:], wT_e[:, 2 * p:2 * p + 2])
                return w

            first = True
            for ly in range(NUM_LAYERS):
                pm = mpsum.tile([128, D], f32, tag="mm", name="mm")
                sT_q = [emit_s(ly, 0), emit_s(ly, 1)]
                if first:
                    # denominators: emitted after the first sT copies so the
                    # Vector FIFO reaches those copies first
                    den = cpool.tile([128, 1], f32, tag="den")
                    nc.vector.tensor_reduce(den[:], adjR_sb[:],
                                            mybir.AxisListType.X, Alu.add)
                    nc.vector.tensor_scalar_add(den[:], den[:], 1.0)
                    recip = cpool.tile([128, 1], f32, tag="recip")
                    nc.vector.reciprocal(recip[:], den[:])
                    recipS = cpool.tile([128, 1], f32, tag="recipS")
                    nc.vector.tensor_scalar_mul(recipS[:], recip[:],
                                                1.0 / W8_SCALE)
                    first = False
                for p in range(NP):
                    if ly == 0 and p + 2 < NP:
                        emit_mask(2 * (p + 2))
                        emit_mask(2 * (p + 2) + 1)
                    if p + 2 < NP:
                        sT_q.append(emit_s(ly, p + 2))
                    if ly == 0 or p >= NP - N_FP8_L2:
                        w = w8[:, 2 * p:2 * p + 2]
                    else:
                        w = get_w2(p)
                    sT = sT_q[p]
                    for l2 in range(2):
                        for kc in range(KC):
                            i = (p * 2 + l2) * KC + kc
                            nc.tensor.matmul(
                                pm[:],
                                lhsT=sT[:, l2, kc, :],
                                rhs=w[:, l2, kc, :],
                                start=(i == 0), stop=(i == L * KC - 1),
                            )
                # relu(msg * recip) -> next h (fp16).  Vector-only in two
                # half ops: a second engine would serialize on the shared
                # PSUM bank anyway and add cross-engine guard latency.
                scl = recipS if ly == 0 else recip
                nc.vector.tensor_scalar(h[ly + 1][:, :256], pm[:, :256],
                                        scl[:], 0.0, Alu.mult, Alu.max)
                nc.vector.tensor_scalar(h[ly + 1][:, 256:], pm[:, 256:],
                                        scl[:], 0.0, Alu.mult, Alu.max)

            # -------- MLP ---------------------------------------------------
            b0_sb = cpool.tile([128, KC], f32, tag="b0")
            nc.sync.dma_start(b0_sb[:], b0_e)
            b1_sb = cpool.tile([128, KC], f32, tag="b1")
            nc.sync.dma_start(b1_sb[:], b1_e)
            w0T_sb = cpool.tile([128, KC, D], f16, tag="w0T")
            nc.sync.dma_start(w0T_sb[:], w0T_e)
            w1T_sb = cpool.tile([128, KC, D], f16, tag="w1T")
            nc.sync.dma_start(w1T_sb[:], w1T_e)
            h_own = h[NUM_LAYERS]
            hT = cpool.tile([128, KC, 128], f16, tag="hT")
            pt = mpsum.tile([128, KC, 128], f16, tag="mm", name="ptr")
            for kc in range(KC):
                nc.tensor.transpose(pt[:, kc, :],
                                    h_own[:, kc * 128:(kc + 1) * 128],
                                    identity[:])
                if kc % 2 == 0:
                    nc.vector.tensor_copy(hT[:, kc], pt[:, kc])
                else:
                    nc.scalar.copy(hT[:, kc], pt[:, kc])

            x1T = cpool.tile([128, KC, 128], f16, tag="x1T")
            px1 = mpsum.tile([128, KC, 128], f32, tag="mm", name="px1")
            for blk in range(KC):
                for kc in range(KC):
                    nc.tensor.matmul(
                        px1[:, blk, :],
                        lhsT=w0T_sb[:, kc, blk * 128:(blk + 1) * 128],
                        rhs=hT[:, kc, :],
                        start=(kc == 0), stop=(kc == KC - 1),
                    )
            # bias+relu after the full matmul stream, split V/ACT
            for blk in range(KC):
                if blk % 2 == 0:
                    nc.vector.tensor_scalar(x1T[:, blk, :], px1[:, blk, :],
                                            b0_sb[:, blk:blk + 1], 0.0,
                                            Alu.add, Alu.max)
                else:
                    nc.scalar.activation(x1T[:, blk, :], px1[:, blk, :],
                                         Act.Relu,
                                         bias=b0_sb[:, blk:blk + 1])

            x2 = cpool.tile([128, KC, 128], f32, tag="x2")
            px2 = mpsum.tile([128, KC, 128], f32, tag="mm", name="px2")
            for blk in range(KC):
                for kc in range(KC):
                    nc.tensor.matmul(
                        px2[:, blk, :],
                        lhsT=w1T_sb[:, kc, blk * 128:(blk + 1) * 128],
                        rhs=x1T[:, kc, :],
                        start=(kc == 0), stop=(kc == KC - 1),
                    )
            for blk in range(KC):
                if blk % 2 == 0:
                    nc.vector.tensor_scalar(x2[:, blk, :], px2[:, blk, :],
                                            b1_sb[:, blk:blk + 1], 0.0,
                                            Alu.add, Alu.max)
                else:
                    nc.scalar.activation(x2[:, blk, :], px2[:, blk, :],
                                         Act.Relu,
                                         bias=b1_sb[:, blk:blk + 1])
                if blk == 1:
                    nc.sync.dma_start(out_e[:, :2], x2[:, :2])
            nc.sync.dma_start(out_e[:, 2:], x2[:, 2:])

    nc.compile()
    return nc


def _get_nc():
    if "nc" not in _CACHE:
        _CACHE["nc"] = _build_nc()
    return _CACHE["nc"]


def kernel(gcn_inputs, word_seq_len, adj_matrix, dep_label_matrix,
           w_params, mlp_w0, mlp_b0, mlp_w1, mlp_b1, **_unused):
    from concourse.bass_utils import run_bass_kernel_spmd

    gcn = np.asarray(gcn_inputs, dtype=np.float32)
    adj = np.asarray(adj_matrix, dtype=np.float32)
    lab = np.asarray(dep_label_matrix)
    w = np.asarray(w_params, dtype=np.float32)
    w0 = np.asarray(mlp_w0, dtype=np.float32)
    w1 = np.asarray(mlp_w1, dtype=np.float32)
    b0 = np.asarray(mlp_b0, dtype=np.float32)
    b1 = np.asarray(mlp_b1, dtype=np.float32)

    # wT[kmod, l, kc, d] = w[l, d, kc*128+kmod]  (shared by all cores)
    wT = w.transpose(0, 2, 1).reshape(L, KC, 128, D).transpose(2, 0, 1, 3)
    wT = np.ascontiguousarray(wT)
    wT16 = wT.astype(np.float16)
    w8 = (wT * W8_SCALE).astype(ml_dtypes.float8_e3m4)
    w0T = np.ascontiguousarray(
        w0.T.reshape(KC, 128, D).transpose(1, 0, 2)).astype(np.float16)
    w1T = np.ascontiguousarray(
        w1.T.reshape(KC, 128, D).transpose(1, 0, 2)).astype(np.float16)
    b0r = np.ascontiguousarray(b0.reshape(KC, 128).T)
    b1r = np.ascontiguousarray(b1.reshape(KC, 128).T)
    labf = lab.astype(np.float32)

    in_maps = []
    for c in range(NCORES):
        in_maps.append({
            "gcn16": gcn[c].astype(np.float16),
            "adjT": np.ascontiguousarray(adj[c].T),
            "labT": np.ascontiguousarray(labf[c].T),
            "adjR": np.ascontiguousarray(adj[c]),
            "w8": w8,
            "wT": wT16,
            "w0T": w0T,
            "w1T": w1T,
            "b0": b0r,
            "b1": b1r,
        })

    nc = _get_nc()
    res = run_bass_kernel_spmd(nc, in_maps, list(range(NCORES)))

    out = np.empty((B, N, D), dtype=np.float32)
    for c in range(NCORES):
        arr = res.results[c]["out"]          # [dmod, blk, i]
        out[c] = np.transpose(arr, (2, 1, 0)).reshape(N, D)
    return out
